# revision 58
# baseline (speedup 1.0000x reference)
"""Trainium2 Bass kernel for ActivationSparseLinear (batched GEMV).

out[b, 0, n] = sum_k x[b, 0, k] * weight[n, k]
  x: (8, 1, 4096) f32, weight: (11008, 4096) f32 -> out: (8, 1, 11008) f32

Strategy (tensor-parallel over out_features, 8 NeuronCores):
  - Each core owns 1376 rows of `weight` and the full (tiny) `x`.
  - Memory-bound on the f32 weight stream (~22.5 MB/core).  The weight
    is DMA'd with an on-the-fly f32->bf16 cast (SWDGE), transposed on
    the TensorEngine via identity matmuls (k onto partitions), bounced
    PSUM->SBUF on DVE/ACT (casting to bf16), then used as the STATIONARY
    operand of per-tile GEMV matmuls whose moving operand is the
    8-column x^T slice (the weight-side traffic rides the LDWEIGHTS
    path, which overlaps in-flight matmuls via the background weight
    buffer), accumulating f32 in PSUM (one bank per 128-row group).
  - No cross-core communication; the host concatenates the 8 shards.
"""

from contextlib import ExitStack

import numpy as np

import concourse.bacc as bacc
import concourse.mybir as mybir
import concourse.tile as tile
from concourse.bass_utils import run_bass_kernel_spmd

B = 8          # batch (seq_len 1 folded away)
K = 4096       # in_features
N = 11008      # out_features
NCORES = 8
N_SHARD = N // NCORES          # 1376 rows per core
KT = K // 128                  # 32 k-tiles
NCHUNK = 512                   # output rows per psum accumulator chunk
KSEG = 2048                    # k columns per weight DMA segment tile

_GRAPH_CACHE = {}


def build_graph() -> bacc.Bacc:
    nc = bacc.Bacc("TRN2", target_bir_lowering=False, debug=False,
                   num_devices=NCORES)
    w = nc.declare_dram_parameter("w", [N_SHARD, K], mybir.dt.float32,
                                  isOutput=False)
    xt = nc.declare_dram_parameter("xt", [128, KT * B], mybir.dt.bfloat16,
                                   isOutput=False)
    ident = nc.declare_dram_parameter("ident", [128, 128], mybir.dt.bfloat16,
                                      isOutput=False)
    out = nc.declare_dram_parameter("out", [N_SHARD, B], mybir.dt.float32,
                                    isOutput=True)

    bf16 = mybir.dt.bfloat16
    f32 = mybir.dt.float32

    # chunk layout: smallest chunk first so the PE's first transposes
    # wait on a 3MB (not 4MB) opening segment; the ragged 96-row tile
    # sits mid-stream (chunk 1); the final chunk is uniform so the
    # end-of-stream chain (last segment -> transposes -> accumulate ->
    # output DMA) stays simple
    chunks = [(0, 384), (384, 480), (864, 512)]
    assert sum(nr for _, nr in chunks) == N_SHARD

    with tile.TileContext(nc) as tc, ExitStack() as ctx:
        const_pool = ctx.enter_context(tc.tile_pool(name="const", bufs=1))
        wn_pool = ctx.enter_context(tc.tile_pool(name="wn", bufs=7))
        wt_pool = ctx.enter_context(tc.tile_pool(name="wt", bufs=8))
        pst_pool = ctx.enter_context(
            tc.tile_pool(name="pst", bufs=4, space="PSUM"))
        psa_pool = ctx.enter_context(
            tc.tile_pool(name="psa", bufs=1, space="PSUM"))
        out_pool = ctx.enter_context(tc.tile_pool(name="outp", bufs=2))

        # constants: x^T (host-pretransposed to [k_in_tile, kt*B]) and the
        # transpose identity, already bf16 on host; HWDGE load keeps the
        # gpsimd SWDGE queue free for the weight stream.
        xt_sb = const_pool.tile([128, KT * B], bf16)
        nc.sync.dma_start(xt_sb[:], xt[:])
        id_sb = const_pool.tile([128, 128], bf16)
        nc.sync.dma_start(id_sb[:], ident[:])

        # segment column counts: big segments minimize per-DMA overhead
        # (the DMA runs several segments ahead of the consumers, so ramp
        # granularity is irrelevant); the last chunk tapers so the final
        # dependency chain after the last byte lands is short
        SEGC = {0: [2048, 2048],
                len(chunks) - 1: [2048, 1024, 512, 256, 256]}

        def seg_dma(w_sb, row0, nrows, jfull, jn, k0, cols):
            if jfull > 0:
                nc.gpsimd.dma_start(
                    w_sb[:, :jfull, :cols],
                    w[row0:row0 + 128 * jfull, k0:k0 + cols].rearrange(
                        "(j p) k -> p j k", p=128))
            if jfull < jn:  # 96-row tail tile
                nc.gpsimd.dma_start(
                    w_sb[:nrows - 128 * jfull, jfull, :cols],
                    w[row0 + 128 * jfull:row0 + nrows, k0:k0 + cols])

        n_copy = 0
        for ci, (row0, nrows) in enumerate(chunks):
            jtiles = [(j, min(128, nrows - j * 128))
                      for j in range((nrows + 127) // 128)]
            jn = len(jtiles)
            jfull = nrows // 128           # number of full 128-row tiles
            # acc[p, j, b] accumulates out rows row0 + j*128 + p; each j
            # lives in its own PSUM bank — an accumulation group's
            # start=True clears has_written for its whole bank, so
            # concurrent groups must not share one
            acc_ps = psa_pool.tile([128, 4, 512], f32, tag="acc")

            # segment tiles: w_seg[p, j, kk] = w[row0+j*128+p, k0+kk]
            ktmap = []
            k0 = 0
            for cols in SEGC.get(ci, [KSEG] * (K // KSEG)):
                w_sb = wn_pool.tile([128, jn, KSEG], bf16, tag="w_sb")
                seg_dma(w_sb, row0, nrows, jfull, jn, k0, cols)
                for kk in range(cols // 128):
                    ktmap.append((w_sb, kk))
                k0 += cols
            assert len(ktmap) == KT

            pend = []
            for kt in range(KT):
                w_sb, kk = ktmap[kt]
                tp_ps = pst_pool.tile([128, NCHUNK], bf16, tag="tp")
                wt_sb = wt_pool.tile([128, NCHUNK], bf16, tag="wt")
                for j, jr in jtiles:
                    # transpose-mode matmul: bf16 PSUM output halves the
                    # bounce-copy volume, letting DVE handle every copy
                    # so the ScalarEngine (and its preamble activation
                    # table load) drops out of the kernel entirely
                    nc.tensor.transpose(
                        tp_ps[:, j * 128:j * 128 + jr],
                        w_sb[:jr, j, kk * 128:(kk + 1) * 128],
                        id_sb[:jr, :jr],
                    )
                nc.vector.tensor_copy(wt_sb[:, :nrows], tp_ps[:, :nrows])
                n_copy += 1

                def emit_gemv(kt_, wt_):
                    for j, jr in jtiles:
                        # GEMV: W^T tile stationary (its LDWEIGHTS hides
                        # behind in-flight matmuls via the background
                        # weight buffer), 8-column x^T moving
                        nc.tensor.matmul(
                            acc_ps[:jr, j, :B],
                            wt_[:, j * 128:j * 128 + jr],
                            xt_sb[:, kt_ * B:(kt_ + 1) * B],
                            start=(kt_ == 0),
                            stop=(kt_ == KT - 1),
                        )

                # two-k-tile software pipelining: the GEMV for tile kt is
                # emitted after tile kt+2's transposes so the PE has work
                # while kt's PSUM->SBUF copy is in flight
                pend.append((kt, wt_sb))
                if len(pend) > 2:
                    emit_gemv(*pend.pop(0))
            for p in pend:
                emit_gemv(*p)
            o_sb = out_pool.tile([128, 4 * B], f32, tag="o")
            nc.vector.tensor_copy(
                o_sb[:, :jn * B].rearrange("p (j b) -> p j b", b=B),
                acc_ps[:, :jn, :B])
            if jfull > 0:
                nc.sync.dma_start(
                    out[row0:row0 + 128 * jfull, :].rearrange(
                        "(j p) b -> p j b", p=128),
                    o_sb[:, :jfull * B].rearrange("p (j b) -> p j b", b=B))
            if jfull < jn:  # 96-row tail tile
                nc.sync.dma_start(
                    out[row0 + 128 * jfull:row0 + nrows, :],
                    o_sb[:nrows - 128 * jfull,
                         jfull * B:(jfull + 1) * B])

    nc.compile()
    return nc


def _get_graph() -> bacc.Bacc:
    if "nc" not in _GRAPH_CACHE:
        _GRAPH_CACHE["nc"] = build_graph()
    return _GRAPH_CACHE["nc"]


def _make_in_maps(x: np.ndarray, weight: np.ndarray):
    x = np.asarray(x, dtype=np.float32).reshape(B, K)
    weight = np.asarray(weight, dtype=np.float32)
    bf16_np = mybir.dt.np(mybir.dt.bfloat16)
    # xt[p, kt*B + b] = x[b, kt*128 + p]
    xt = np.ascontiguousarray(
        x.reshape(B, KT, 128).transpose(2, 1, 0).reshape(128, KT * B)
    ).astype(bf16_np)
    ident = np.eye(128, dtype=np.float32).astype(bf16_np)
    in_maps = []
    for c in range(NCORES):
        w_shard = np.ascontiguousarray(
            weight[c * N_SHARD:(c + 1) * N_SHARD, :])
        in_maps.append({"w": w_shard, "xt": xt, "ident": ident})
    return in_maps


def _run(x: np.ndarray, weight: np.ndarray, trace: bool = False):
    nc = _get_graph()
    in_maps = _make_in_maps(x, weight)
    res = run_bass_kernel_spmd(nc, in_maps, core_ids=list(range(NCORES)),
                               trace=trace)
    out = np.empty((B, 1, N), dtype=np.float32)
    for c in range(NCORES):
        out[:, 0, c * N_SHARD:(c + 1) * N_SHARD] = res.results[c]["out"].T
    return out, res


def kernel(x: np.ndarray, weight: np.ndarray) -> np.ndarray:
    out, _ = _run(x, weight, trace=False)
    return out
